# revision 1
# baseline (speedup 1.0000x reference)
"""Trainium2 Bass kernel: CodeEncoder attention pooling.

Math (per (b,v) bag): gather 64 embeddings [C=64, D=256] from a 20000x256
table, score each code with score = W2 @ tanh(W1 @ e + b1) (+b2, dropped:
softmax is shift-invariant), masked softmax over the 64 codes (c < length,
else -1e9), output = sum_c attn[c] * e[c].

Key structure: the score of a code depends only on its vocab id, so each
core computes score_table[20000] once (tiny MLP over the transposed table)
and per-code scores are a scalar gather — the per-code MLP disappears.

Sharding: data-parallel over batch B=64 -> 8 batches per core on 8 cores.
Per core: 400 bags, 25600 gathered rows.

Pipeline per core (f16 data, f32 accumulation):
  1. dma_gather: 25600 table rows (512B each) -> SBUF emb [128p, 200, 256],
     code i lands on partition i%128, block i//128 (so bag pair (2g,2g+1)
     occupies the 128 partitions of block g).
  2. Score table: for v-chunks of the transposed table tableT [2,128,20000],
     h.T = tanh(W1 @ tableT + b1) on PE+ACT, w = W2rep @ h.T (W2 replicated
     128x so every partition holds the full score row) -> score_rep
     [128, 20000] f16 in SBUF.
  3. ap_gather (gpsimd): per-code scores; 8 Q7 groups each gather their own
     3200 codes. Redistribute to [bag-on-partition, 64] layout via 8 tiny
     SBUF->SBUF DMAs (100 bags per tile, 4 tiles).
  4. Masked softmax along free axis; PE-transpose attn -> [64, 128]; build
     block-diagonal lhsT [128, 2*50] per tile with 2 strided copies.
  5. Pooling: per bag pair one matmul lhsT=[128,2] block, rhs=emb block
     [128,256] -> psum rows 2g..2g+1; 50 pairs fill a [100,256] psum tile;
     one DVE copy + one output DMA per 100 bags.
"""

import os
import sys

if "/opt/trn_rl_repo" not in sys.path:
    sys.path.insert(0, "/opt/trn_rl_repo")

from contextlib import ExitStack

import numpy as np

PHASE = int(os.environ.get("KERNEL_PHASE", "5"))

B, V, C = 64, 50, 64
NUM_CODE, D, H = 20000, 256, 128
NCORES = 8
BPC = B // NCORES          # batches per core
BAGS = BPC * V             # 400 bags per core
CODES = BAGS * C           # 25600 codes per core
NGATHER = 8                # dma_gather calls per core
GCHUNK = CODES // NGATHER  # 3200 idxs per dma_gather
GBLK = GCHUNK // 128       # 25 column-blocks per gather call
NBLK = CODES // 128        # 200 column-blocks total
TILE_BAGS = (128, 128, 128, 16)  # bags per softmax/pooling tile
NT = len(TILE_BAGS)
VCH = 2000                 # table columns per DMA chunk
NSL = 500                  # matmul N slice (<=512 f32 psum)

_cache = {}


def _build_program():
    import concourse.bass as bass
    import concourse.tile as tile
    from concourse import bacc, mybir

    f16 = mybir.dt.float16
    f32 = mybir.dt.float32
    i16 = mybir.dt.int16

    nc = bacc.Bacc("TRN2", target_bir_lowering=False, debug=False,
                   num_devices=NCORES)

    table_d = nc.dram_tensor("table", [NUM_CODE, D], f16, kind="ExternalInput")
    tableT_d = nc.dram_tensor("tableT", [D, NUM_CODE], f16,
                              kind="ExternalInput")
    w1t_d = nc.dram_tensor("w1t", [D, H], f16, kind="ExternalInput")
    w2rep_d = nc.dram_tensor("w2rep", [H, 128], f16, kind="ExternalInput")
    b1_d = nc.dram_tensor("b1", [H, 1], f32, kind="ExternalInput")
    gidx_d = nc.dram_tensor("gidx", [128, CODES // 16], i16,
                            kind="ExternalInput")
    aidx_d = nc.dram_tensor("aidx", [128, GCHUNK // 16], i16,
                            kind="ExternalInput")
    par_d = nc.dram_tensor("par", [128, GCHUNK], mybir.dt.uint8,
                            kind="ExternalInput")
    lens_d = nc.dram_tensor("lens", [128, NT], f32, kind="ExternalInput")
    cvals_d = nc.dram_tensor("cvals", [128, C], f32, kind="ExternalInput")
    ident_d = nc.dram_tensor("ident", [128, 128], f16, kind="ExternalInput")
    out_d = nc.dram_tensor("out", [BAGS, D], f32, kind="ExternalOutput")

    with tile.TileContext(nc) as tc, ExitStack() as ctx:
        const = ctx.enter_context(tc.tile_pool(name="const", bufs=1))
        tabp = ctx.enter_context(tc.tile_pool(name="tabp", bufs=2))
        hp = ctx.enter_context(tc.tile_pool(name="hp", bufs=3))
        soft = ctx.enter_context(tc.tile_pool(name="soft", bufs=2))
        blkp = ctx.enter_context(tc.tile_pool(name="blkp", bufs=NT))
        outp = ctx.enter_context(tc.tile_pool(name="outp", bufs=2))
        ph_p = ctx.enter_context(tc.tile_pool(name="ph", bufs=2, space="PSUM"))
        pw_p = ctx.enter_context(tc.tile_pool(name="pw", bufs=2, space="PSUM"))
        ptr_p = ctx.enter_context(tc.tile_pool(name="ptr", bufs=2, space="PSUM"))
        ppool_p = ctx.enter_context(tc.tile_pool(name="ppool", bufs=2,
                                                 space="PSUM"))

        # --- constant / input uploads (HWDGE) ---
        w1t_ap = w1t_d.ap()
        tableT_ap = tableT_d.ap()
        w1t_sb = const.tile([128, 2, H], f16)
        nc.sync.dma_start(w1t_sb[:, 0, :], w1t_ap[0:128, :])
        nc.sync.dma_start(w1t_sb[:, 1, :], w1t_ap[128:256, :])
        w2rep_sb = const.tile([H, 128], f16)
        nc.sync.dma_start(w2rep_sb[:], w2rep_d.ap())
        b1_sb = const.tile([H, 1], f32)
        nc.sync.dma_start(b1_sb[:], b1_d.ap())
        gidx_sb = const.tile([128, CODES // 16], i16)
        nc.sync.dma_start(gidx_sb[:], gidx_d.ap())
        aidx_sb = const.tile([128, GCHUNK // 16], i16)
        nc.sync.dma_start(aidx_sb[:], aidx_d.ap())
        par_sb = const.tile([128, GCHUNK], mybir.dt.uint8)
        nc.sync.dma_start(par_sb[:], par_d.ap())
        lens_sb = const.tile([128, NT], f32)
        nc.sync.dma_start(lens_sb[:], lens_d.ap())
        cvals_sb = const.tile([128, C], f32)
        nc.sync.dma_start(cvals_sb[:], cvals_d.ap())
        ident_sb = const.tile([128, 128], f16)
        nc.sync.dma_start(ident_sb[:], ident_d.ap())

        # --- embedding gather: 512B rows; the SWDGE gather ucode crashes
        # beyond ~1.5k idxs per call, so chunk at 1024 ---
        DG_N = 1024
        emb_sb = const.tile([128, NBLK, D], f16)
        if PHASE < 5:
            probe = const.tile([128, D], f32)
            nc.vector.memset(probe[:], 0.0)
            nc.sync.dma_start(out_d.ap()[0:128, :], probe[:])
        for k in range(CODES // DG_N):
            nc.gpsimd.dma_gather(
                emb_sb[:, k * (DG_N // 128):(k + 1) * (DG_N // 128), :],
                table_d.ap(),
                gidx_sb[:, k * (DG_N // 16):(k + 1) * (DG_N // 16)],
                DG_N, DG_N, D,
            )

        # --- score table build ---
        score_rep = const.tile([128, NUM_CODE], f16)
        ncp = 0
        for ci in range(NUM_CODE // VCH if PHASE >= 2 else 0):
            tab_t = tabp.tile([128, 2, VCH], f16)
            sl = slice(ci * VCH, (ci + 1) * VCH)
            nc.sync.dma_start(tab_t[:, 0, :], tableT_ap[0:128, sl])
            nc.sync.dma_start(tab_t[:, 1, :], tableT_ap[128:256, sl])
            for ni in range(VCH // NSL):
                nsl = slice(ni * NSL, (ni + 1) * NSL)
                gsl = slice(ci * VCH + ni * NSL, ci * VCH + (ni + 1) * NSL)
                ph = ph_p.tile([128, NSL], f32)
                nc.tensor.matmul(ph[:], w1t_sb[:, 0, :], tab_t[:, 0, nsl],
                                 start=True, stop=False)
                nc.tensor.matmul(ph[:], w1t_sb[:, 1, :], tab_t[:, 1, nsl],
                                 start=False, stop=True)
                h_sb = hp.tile([128, NSL], f16)
                nc.scalar.activation(h_sb[:], ph[:],
                                     mybir.ActivationFunctionType.Tanh,
                                     bias=b1_sb[:], scale=1.0)
                pw = pw_p.tile([128, NSL], f32)
                nc.tensor.matmul(pw[:], w2rep_sb[:], h_sb[:],
                                 start=True, stop=True)
                # every psum partition holds the same scores; cast to f16
                if ncp % 2 == 0:
                    nc.vector.tensor_copy(score_rep[:, gsl], pw[:])
                else:
                    nc.scalar.copy(score_rep[:, gsl], pw[:])
                ncp += 1

        # --- per-code score gather (8 Q7 groups, own chunks) ---
        # ap_gather moves 4-byte units, so gather f16 score PAIRS (d=2,
        # idx = code//2) and select the correct half with a parity mask.
        # the Q7 ap_gather ucode corrupts the tail beyond ~256 idxs/call
        pairs = const.tile([128, GCHUNK, 2], f16)
        off = 0 if PHASE >= 3 else GCHUNK
        while off < GCHUNK:
            n = min(256, GCHUNK - off)
            nc.gpsimd.ap_gather(
                pairs[:, off:off + n, :],
                score_rep[:].rearrange("p (n d) -> p n d", d=2),
                aidx_sb[:, off // 16:(off + n) // 16],
                channels=128, num_elems=NUM_CODE // 2, d=2, num_idxs=n,
            )
            off += n
        scoresg = const.tile([128, GCHUNK], f16)
        if PHASE >= 3:
            nc.vector.tensor_copy(
            scoresg[:].rearrange("p (n d) -> p n d", d=1),
            pairs[:, :, 0:1])
            nc.vector.copy_predicated(
                scoresg[:].rearrange("p (n d) -> p n d", d=1),
                par_sb[:].rearrange("p (n d) -> p n d", d=1),
                pairs[:, :, 1:2])

        # --- redistribute group-row scores to [bag, c] tiles ---
        # group k holds bags [50k, 50k+50) on partition 16k; bag 128t+p
        # lands on partition p of tile t (split DMAs at tile crossings)
        scores_sb = const.tile([128, NT, C], f16)
        nc.vector.memset(scores_sb[:], 0)
        for k in range(NGATHER if PHASE >= 3 else 0):
            src = scoresg[16 * k:16 * k + 1, :].rearrange(
                "p (b c) -> p b c", c=C)
            t0, p0 = divmod(50 * k, 128)
            n1 = min(50, 128 - p0)
            nc.sync.dma_start(scores_sb[p0:p0 + n1, t0, :],
                              src[0:1, 0:n1, :])
            if n1 < 50:
                nc.sync.dma_start(scores_sb[0:50 - n1, t0 + 1, :],
                                  src[0:1, n1:50, :])

        # --- masked softmax per tile of 100 bags ---
        for t in range(NT if PHASE >= 4 else 0):
            m01 = soft.tile([128, C], f32, tag="m01")
            nc.vector.tensor_scalar(m01[:], cvals_sb[:], lens_sb[:, t:t + 1],
                                    None, mybir.AluOpType.is_lt)
            madd = soft.tile([128, C], f32, tag="madd")
            nc.vector.tensor_scalar(madd[:], m01[:], 1.0, 1e9,
                                    mybir.AluOpType.subtract,
                                    mybir.AluOpType.mult)
            s32 = soft.tile([128, C], f32, tag="s32")
            nc.vector.tensor_copy(s32[:], scores_sb[:, t, :])
            wm = soft.tile([128, C], f32, tag="wm")
            nc.vector.tensor_mul(wm[:], s32[:], m01[:])
            nc.vector.tensor_add(wm[:], wm[:], madd[:])
            nmx = soft.tile([128, 1], f32, tag="nmx")
            nc.vector.tensor_reduce(nmx[:], wm[:], mybir.AxisListType.X,
                                    mybir.AluOpType.max, negate=True)
            ex = soft.tile([128, C], f32, tag="ex")
            sm = soft.tile([128, 1], f32, tag="sm")
            nc.scalar.activation(ex[:], wm[:],
                                 mybir.ActivationFunctionType.Exp,
                                 bias=nmx[:], scale=1.0, accum_out=sm[:])
            rs = soft.tile([128, 1], f32, tag="rs")
            nc.vector.reciprocal(rs[:], sm[:])
            attn = soft.tile([128, C], f16, tag="attn")
            nc.vector.tensor_scalar(attn[:], ex[:], rs[:], None,
                                    mybir.AluOpType.mult)
            # transpose attn -> [c, bag]
            ptr = ptr_p.tile([C, 128], f16)
            nc.tensor.transpose(ptr[:], attn[:], ident_sb[:])
            attnT = soft.tile([C, 128], f16, tag="attnT")
            nc.vector.tensor_copy(attnT[:], ptr[:])
            # Pooling. PE output base partitions must be 32-aligned, so
            # pairs are grouped 16 per 32-row psum stripe: pair slot s of
            # group j uses lhsT [128, 32] with only columns 2s (rows 0:64 =
            # even bag's attn) and 2s+1 (rows 64:128 = odd bag) nonzero;
            # the 16 matmuls accumulate into psum[32j:32j+32].
            # The (s -> column 2s) structure is a diagonal, built with two
            # strided copies: flat offset s*32 + 2s = 34s.
            nb = TILE_BAGS[t]
            ppool = ppool_p.tile([128, D], f32)
            at_ap = attnT[:]
            for j in range((nb + 31) // 32):
                npair = min(16, nb // 2 - 16 * j)
                blockT = blkp.tile([128, 16, 32], f16)
                nc.vector.memset(blockT[:], 0)
                bt_ap = blockT[:]
                dst_even = bass.AP(bt_ap.tensor, bt_ap.offset,
                                   [[512, C], [34, npair], [1, 1]])
                dst_odd = bass.AP(bt_ap.tensor, bt_ap.offset + C * 512 + 1,
                                  [[512, C], [34, npair], [1, 1]])
                src_even = bass.AP(at_ap.tensor, at_ap.offset + 32 * j,
                                   [[128, C], [2, npair], [1, 1]])
                src_odd = bass.AP(at_ap.tensor, at_ap.offset + 32 * j + 1,
                                  [[128, C], [2, npair], [1, 1]])
                nc.vector.tensor_copy(dst_even, src_even)
                nc.vector.tensor_copy(dst_odd, src_odd)
                for s in range(npair):
                    nc.tensor.matmul(ppool[32 * j:32 * j + 32, :],
                                     blockT[:, s, :],
                                     emb_sb[:, 64 * t + 16 * j + s, :],
                                     start=(s == 0), stop=(s == npair - 1),
                                     tile_position=(0, 32 * j))
            out_sb = outp.tile([128, D], f32)
            nc.vector.tensor_copy(out_sb[0:nb, :], ppool[0:nb, :])
            nc.sync.dma_start(out_d.ap()[128 * t:128 * t + nb, :],
                              out_sb[0:nb, :])

    nc.compile()
    return nc


def _wrap16(idx_flat):
    """dma_gather/ap_gather index layout: idx i -> partition i%16, slot
    i//16."""
    n = idx_flat.shape[0]
    return idx_flat.reshape(n // 16, 16).T.copy()  # [16, n//16]


def _prep_shared(embed_table, W1, b1, W2):
    tab16 = embed_table.astype(np.float16)                    # [20000, 256]
    tableT = np.ascontiguousarray(tab16.T)                    # [256, 20000]
    w1t = np.ascontiguousarray(W1.astype(np.float16).T)       # [256, 128]
    w2rep = np.repeat(W2.astype(np.float16).reshape(H, 1), 128, axis=1)
    b1c = np.ascontiguousarray(b1.astype(np.float32).reshape(H, 1))
    cvals = np.broadcast_to(np.arange(C, dtype=np.float32), (128, C)).copy()
    ident = np.eye(128, dtype=np.float16)
    return dict(table=tab16, tableT=tableT, w1t=w1t, w2rep=w2rep, b1=b1c,
                cvals=cvals, ident=ident)


def build_in_maps(input_code, length_code, shared):
    in_maps = []
    for core in range(NCORES):
        bs = slice(core * BPC, (core + 1) * BPC)
        codes = input_code[bs].reshape(-1).astype(np.int16)     # [25600]
        gidx = np.tile(_wrap16(codes), (8, 1))                  # [128, 1600]
        aidx = np.concatenate(
            [_wrap16(codes[k * GCHUNK:(k + 1) * GCHUNK] // 2) for k in
             range(NGATHER)], axis=0)                           # [128, 200]
        par = np.concatenate(
            [np.broadcast_to(
                (codes[k * GCHUNK:(k + 1) * GCHUNK] % 2).astype(np.uint8),
                (16, GCHUNK)) for k in range(NGATHER)], axis=0)  # [128, 3200]
        lens = np.full((128, NT), C, dtype=np.float32)
        lv = length_code[bs].reshape(-1).astype(np.float32)     # [400]
        for t in range(NT):
            lens[:TILE_BAGS[t], t] = lv[128 * t:128 * t + TILE_BAGS[t]]
        in_maps.append(dict(shared, gidx=gidx, aidx=aidx, par=par, lens=lens))
    return in_maps


def kernel(input_code, length_code, embed_table, W1, b1, W2, b2):
    from concourse.bass_utils import run_bass_kernel_spmd

    if "nc" not in _cache:
        _cache["nc"] = _build_program()
    nc = _cache["nc"]

    shared = _prep_shared(np.asarray(embed_table), np.asarray(W1),
                          np.asarray(b1), np.asarray(W2))
    input_code = np.asarray(input_code)
    length_code = np.asarray(length_code)

    in_maps = build_in_maps(input_code, length_code, shared)
    res = run_bass_kernel_spmd(nc, in_maps, core_ids=list(range(NCORES)))
    outs = [res.results[c]["out"].reshape(BPC, V, D) for c in range(NCORES)]
    return np.concatenate(outs, axis=0)



# revision 5
# speedup vs baseline: 1.7179x; 1.7179x over previous
"""Trainium2 Bass kernel: CodeEncoder attention pooling, histogram form.

Math per bag: out = sum_c softmax(score(idx_c))_c * table[idx_c]. Scores
depend only on the vocab id (score = W2 tanh(W1 e + b1); b2 cancels in
softmax), so with per-bag vocab counts Cnt[bag,v] (host-built from the
indices, valid codes only):

    g(v)    = exp(score_v)                      (device, score-table MLP)
    num     = Cnt @ (g*table)   [BAGS, 256]     (device, dense matmul)
    Z       = Cnt @ g           [BAGS]          (ones column of the rhs)
    out     = num / Z

This removes both data-dependent gathers (the SWDGE dma_gather and the
Q7 ap_gather dominated the old runtime). The count matrix is index prep,
computed host-side and streamed as f16.

Length-0 bags (softmax over all -1e9 -> uniform 1/64 over all 64 codes)
don't fit the weighted form: they are pooled by a small dma_gather of
their 64*32 rows + one block-diagonal mean matmul into padded output
rows; the host maps those rows back.

Sharding: data-parallel over batch, 8 batches/core on 8 cores.
Per-core pipeline: score MLP over tableT slices (PE+ACT) -> g [128,160]
via per-128-block W2 matmuls (v on psum partitions, no transposes) ->
exp -> scale rhs chunks by g (DVE) -> 160x4 matmuls accumulate
[bags<=128, 260] psum over chunks -> divide by Z column -> out.
"""

import sys

if "/opt/trn_rl_repo" not in sys.path:
    sys.path.insert(0, "/opt/trn_rl_repo")

from contextlib import ExitStack

import numpy as np

B, V, C = 64, 50, 64
NUM_CODE, D, H = 20000, 256, 128
NCORES = 8
BPC = B // NCORES          # batches per core
BAGS = BPC * V             # 400 bags per core
VP = 20480                 # padded vocab (160 chunks of 128, 40 slices of 512)
NCH = VP // 128            # 160 vocab chunks
NSL = 512                  # score-MLP slice (one f32 psum bank)
TSL = 2048                 # tableT columns per DMA
NW = D + 4                 # rhs width: 256 emb + ones col + 3 pad
MT = (128, 128, 128, 16)   # bag m-tiles
CGRP = 10                  # count-matrix chunks per DMA
ESLOT = 32                 # length-0 bag slots
ECODES = ESLOT * C         # 2048 gathered rows for the epilogue
OUTR = BAGS + ESLOT        # padded output rows

_cache = {}


def _build_program():
    import concourse.bass as bass  # noqa: F401
    import concourse.tile as tile
    from concourse import bacc, mybir

    f16 = mybir.dt.float16
    f32 = mybir.dt.float32
    i16 = mybir.dt.int16

    nc = bacc.Bacc("TRN2", target_bir_lowering=False, debug=False,
                   num_devices=NCORES)

    table_d = nc.dram_tensor("table", [NUM_CODE, D], f16, kind="ExternalInput")
    tabt_d = nc.dram_tensor("tabt", [D, VP], f16, kind="ExternalInput")
    rhsc_d = nc.dram_tensor("rhsc", [128, NCH * NW], f16, kind="ExternalInput")
    cnt_d = nc.dram_tensor("cnt", [128, NCH * BAGS], f16, kind="ExternalInput")
    w1t_d = nc.dram_tensor("w1t", [D, H], f16, kind="ExternalInput")
    w2c_d = nc.dram_tensor("w2c", [H, 1], f16, kind="ExternalInput")
    b1_d = nc.dram_tensor("b1", [H, 1], f32, kind="ExternalInput")
    epool_d = nc.dram_tensor("epool", [128, 16 * 32], f16, kind="ExternalInput")
    egidx_d = nc.dram_tensor("egidx", [128, ECODES // 16], i16,
                             kind="ExternalInput")
    out_d = nc.dram_tensor("out", [OUTR, D], f32, kind="ExternalOutput")

    with tile.TileContext(nc) as tc, ExitStack() as ctx:
        const = ctx.enter_context(tc.tile_pool(name="const", bufs=1))
        tabp = ctx.enter_context(tc.tile_pool(name="tabp", bufs=3))
        hp = ctx.enter_context(tc.tile_pool(name="hp", bufs=3))
        cp = ctx.enter_context(tc.tile_pool(name="cp", bufs=3))
        tgp = ctx.enter_context(tc.tile_pool(name="tgp", bufs=8))
        outp = ctx.enter_context(tc.tile_pool(name="outp", bufs=2))
        php = ctx.enter_context(tc.tile_pool(name="ph", bufs=2, space="PSUM"))
        gp = ctx.enter_context(tc.tile_pool(name="gp", bufs=1, space="PSUM"))
        mp = ctx.enter_context(tc.tile_pool(name="mp", bufs=1, space="PSUM"))
        epp = ctx.enter_context(tc.tile_pool(name="epp", bufs=1, space="PSUM"))

        # --- constants ---
        w1t_sb = const.tile([128, 2, H], f16)
        nc.sync.dma_start(w1t_sb[:, 0, :], w1t_d.ap()[0:128, :])
        nc.sync.dma_start(w1t_sb[:, 1, :], w1t_d.ap()[128:256, :])
        w2c_sb = const.tile([H, 1], f16)
        nc.sync.dma_start(w2c_sb[:], w2c_d.ap())
        b1_sb = const.tile([H, 1], f32)
        nc.sync.dma_start(b1_sb[:], b1_d.ap())
        epool_sb = const.tile([128, 16, 32], f16)
        nc.sync.dma_start(epool_sb[:].rearrange("p a b -> p (a b)"),
                          epool_d.ap())
        egidx_sb = const.tile([128, ECODES // 16], i16)
        nc.sync.dma_start(egidx_sb[:], egidx_d.ap())

        # rhs = [table | 1 | pad] blocked [128, 160, 260]
        rhsc_sb = const.tile([128, NCH, NW], f16)
        for a in range(0, NCH, 20):
            nc.sync.dma_start(
                rhsc_sb[:, a:a + 20, :].rearrange("p a b -> p (a b)"),
                rhsc_d.ap()[:, a * NW:(a + 20) * NW])

        # --- length-0 epilogue gather (gpsimd is otherwise idle) ---
        egat = const.tile([128, ECODES // 128, D], f16)
        for k in range(ECODES // 1024):
            nc.gpsimd.dma_gather(
                egat[:, k * 8:(k + 1) * 8, :], table_d.ap(),
                egidx_sb[:, k * 64:(k + 1) * 64], 1024, 1024, D)

        # --- score table: g = exp(W2 tanh(W1 tabT + b1)), v on partitions ---
        g_ps = gp.tile([128, NCH], f32)
        for ti in range(VP // TSL):
            tab_t = tabp.tile([128, 2, TSL], f16)
            sl = slice(ti * TSL, (ti + 1) * TSL)
            nc.sync.dma_start(tab_t[:, 0, :], tabt_d.ap()[0:128, sl])
            nc.sync.dma_start(tab_t[:, 1, :], tabt_d.ap()[128:256, sl])
            for sub in range(TSL // NSL):
                ssl = slice(sub * NSL, (sub + 1) * NSL)
                ph = php.tile([128, NSL], f32)
                nc.tensor.matmul(ph[:], w1t_sb[:, 0, :], tab_t[:, 0, ssl],
                                 start=True, stop=False)
                nc.tensor.matmul(ph[:], w1t_sb[:, 1, :], tab_t[:, 1, ssl],
                                 start=False, stop=True)
                h1 = hp.tile([128, NSL], f16)
                nc.scalar.activation(h1[:], ph[:],
                                     mybir.ActivationFunctionType.Tanh,
                                     bias=b1_sb[:], scale=1.0)
                # W2: v-block on psum partitions, one column per chunk
                for k in range(NSL // 128):
                    j = (ti * (TSL // NSL) + sub) * (NSL // 128) + k
                    nc.tensor.matmul(g_ps[:, j:j + 1],
                                     h1[:, k * 128:(k + 1) * 128], w2c_sb[:],
                                     start=True, stop=True)
        g_sb = const.tile([128, NCH], f32)
        nc.scalar.activation(g_sb[:], g_ps[:],
                             mybir.ActivationFunctionType.Exp)

        # --- main: accumulate [bags, 260] over 160 vocab chunks ---
        mtiles = []
        o = 0
        for m in MT:
            mtiles.append((o, m))
            o += m
        mps = [mp.tile([128, NW], f32, name=f"mps{t}", tag=f"mps{t}")
               for t in range(len(MT))]
        for grp in range(NCH // CGRP):
            ct = cp.tile([128, CGRP, BAGS], f16)
            nc.sync.dma_start(
                ct[:].rearrange("p a b -> p (a b)"),
                cnt_d.ap()[:, grp * CGRP * BAGS:(grp + 1) * CGRP * BAGS])
            for jj in range(CGRP):
                j = grp * CGRP + jj
                tg = tgp.tile([128, NW], f16)
                nc.vector.tensor_scalar(tg[:], rhsc_sb[:, j, :],
                                        g_sb[:, j:j + 1], None,
                                        mybir.AluOpType.mult)
                for t, (o, m) in enumerate(mtiles):
                    nc.tensor.matmul(mps[t][0:m, :], ct[:, jj, o:o + m],
                                     tg[:], start=(j == 0), stop=(j == NCH - 1))

        # --- length-0 epilogue: mean over 64 codes per slot ---
        eps = epp.tile([32, D], f32)
        for gb in range(16):
            nc.tensor.matmul(eps[:], epool_sb[:, gb, :], egat[:, gb, :],
                             start=(gb == 0), stop=(gb == 15))
        eout = outp.tile([32, D], f32)
        nc.vector.tensor_copy(eout[:], eps[:])
        nc.sync.dma_start(out_d.ap()[BAGS:OUTR, :], eout[:])

        # --- normalize and store ---
        for t, (o, m) in enumerate(mtiles):
            rz = hp.tile([128, 1], f32, tag=f"rz{t}")
            nc.vector.reciprocal(rz[0:m], mps[t][0:m, D:D + 1])
            osb = outp.tile([128, D], f32)
            nc.vector.tensor_scalar(osb[0:m, :], mps[t][0:m, 0:D], rz[0:m],
                                    None, mybir.AluOpType.mult)
            nc.sync.dma_start(out_d.ap()[o:o + m, :], osb[0:m, :])

    nc.compile()
    return nc


def _wrap16(idx_flat):
    n = idx_flat.shape[0]
    return idx_flat.reshape(n // 16, 16).T.copy()


def _prep_shared(embed_table, W1, b1, W2):
    tab16 = embed_table.astype(np.float16)                    # [20000, 256]
    tabt = np.zeros((D, VP), np.float16)
    tabt[:, :NUM_CODE] = tab16.T
    rhsc = np.zeros((VP, NW), np.float16)
    rhsc[:NUM_CODE, :D] = tab16
    rhsc[:, D] = 1.0
    rhsc = np.ascontiguousarray(
        rhsc.reshape(NCH, 128, NW).transpose(1, 0, 2)).reshape(128, NCH * NW)
    w1t = np.ascontiguousarray(W1.astype(np.float16).T)       # [256, 128]
    w2c = np.ascontiguousarray(W2.astype(np.float16).reshape(H, 1))
    b1c = np.ascontiguousarray(b1.astype(np.float32).reshape(H, 1))
    epool = np.zeros((128, 16, 32), np.float16)
    for g in range(16):
        epool[0:64, g, 2 * g] = 1.0 / C
        epool[64:128, g, 2 * g + 1] = 1.0 / C
    epool = epool.reshape(128, 16 * 32)
    return dict(table=tab16, tabt=tabt, rhsc=rhsc, w1t=w1t, w2c=w2c, b1=b1c,
                epool=epool)


def build_in_maps(input_code, length_code, shared):
    in_maps = []
    len0_lists = []
    for core in range(NCORES):
        bs = slice(core * BPC, (core + 1) * BPC)
        codes = input_code[bs].reshape(BAGS, C).astype(np.int64)
        lens = length_code[bs].reshape(BAGS).astype(np.int64)
        valid = np.arange(C)[None, :] < lens[:, None]          # [400, 64]
        bb, cc = np.nonzero(valid)
        cnt = np.zeros((VP, BAGS), np.float32)
        np.add.at(cnt, (codes[bb, cc], bb), 1.0)
        cnt = np.ascontiguousarray(
            cnt.reshape(NCH, 128, BAGS).transpose(1, 0, 2)
        ).astype(np.float16).reshape(128, NCH * BAGS)
        len0 = np.nonzero(lens == 0)[0][:ESLOT]
        ecodes = np.zeros(ECODES, np.int16)
        for s, b in enumerate(len0):
            ecodes[s * C:(s + 1) * C] = codes[b]
        egidx = np.tile(_wrap16(ecodes), (8, 1))               # [128, 128]
        len0_lists.append(len0)
        in_maps.append(dict(shared, cnt=cnt, egidx=egidx))
    return in_maps, len0_lists


def kernel(input_code, length_code, embed_table, W1, b1, W2, b2):
    from concourse.bass_utils import run_bass_kernel_spmd

    if "nc" not in _cache:
        _cache["nc"] = _build_program()
    nc = _cache["nc"]

    shared = _prep_shared(np.asarray(embed_table), np.asarray(W1),
                          np.asarray(b1), np.asarray(W2))
    input_code = np.asarray(input_code)
    length_code = np.asarray(length_code)

    in_maps, len0_lists = build_in_maps(input_code, length_code, shared)
    res = run_bass_kernel_spmd(nc, in_maps, core_ids=list(range(NCORES)))
    outs = []
    for c in range(NCORES):
        full = res.results[c]["out"]
        o = full[:BAGS].copy()
        for s, b in enumerate(len0_lists[c]):
            o[b] = full[BAGS + s]
        outs.append(o.reshape(BPC, V, D))
    return np.concatenate(outs, axis=0)
